# revision 1
# baseline (speedup 1.0000x reference)
"""Erosion (5x5 sliding-window min, geodesic border pad 1e4) on TRN2.

Layout: partition p holds rows 8p-2 .. 8p+9 of one image as 12
free-dim segments (halo -2,-1 | main 0..7 | halo +8,+9), each 1028
cols (2-col pads). The 4-row halo is re-read from DRAM via strided,
partition-aligned DMAs (SBUF->SBUF partition-shifted copies measure
~50 GB/s and must be avoided). Vertical pass = 3 shifted
tensor_tensor(min) along the segment axis, horizontal pass = 3 shifted
TTs within segments — all free-dim, no cross-partition traffic.

Column chunks of 128 for intermediates; the final op of each chunk
writes into a full-width per-image output tile so the store is ONE
DMA of 4KB runs per image (per-chunk stores with 1KB runs cost ~105us
in HWDGE descriptor generation). Loads ride the sync queue, stores the
scalar queue; t1/t2 pools triple-buffered.
"""

import numpy as np

import concourse.bacc as bacc
import concourse.mybir as mybir
import concourse.tile as tile
from concourse.bass_utils import run_bass_kernel_spmd

B, H, W = 32, 1024, 1024
N_CORES = 8
PER_CORE = B // N_CORES     # 4 images per core
PX = 2
PAD_VAL = 1e4
F32 = mybir.dt.float32
MIN = mybir.AluOpType.min

KR = 8                      # output rows per partition (128*8 = 1024)
SEGS = KR + 2 * PX          # 12 segments per partition
WP = W + 2 * PX             # 1028 padded width
CW = 128                    # output cols per chunk
CWH = CW + 2 * PX           # 132
N_CC = W // CW              # 8

_CACHE = {}


def build_nc(repeat: int = 1):
    nc = bacc.Bacc("TRN2", debug=False, num_devices=N_CORES)
    x = nc.dram_tensor("mask", [PER_CORE, H, W], F32, kind="ExternalInput").ap()
    y = nc.dram_tensor("out", [PER_CORE, H, W], F32, kind="ExternalOutput").ap()

    with tile.TileContext(nc) as tc:
        with (
            tc.tile_pool(name="const", bufs=1) as cpool,
            tc.tile_pool(name="xp", bufs=2) as xpool,
            tc.tile_pool(name="t1", bufs=3) as t1p,
            tc.tile_pool(name="t2", bufs=3) as t2p,
            tc.tile_pool(name="vp", bufs=2) as vpool,
            tc.tile_pool(name="op", bufs=1) as opool,
        ):
            # 1e4 source for row-pad fills (memset can't start at
            # partition 127; DMA is exempt from start-partition rules)
            cpad = cpool.tile([128, 2 * WP], F32)
            nc.vector.memset(cpad[:, :], PAD_VAL)

            for rep in range(repeat):
                for img in range(PER_CORE):
                    xt = xpool.tile([128, SEGS * WP], F32, tag="x")
                    x3 = xt[:, :].rearrange("p (s c) -> p s c", s=SEGS)

                    # column pads (all segments)
                    nc.vector.memset(x3[:, :, 0:PX], PAD_VAL)
                    nc.vector.memset(x3[:, :, W + PX : WP], PAD_VAL)
                    # row pads: partition 0 segs 0,1 / partition 127 segs 10,11
                    nc.sync.dma_start(
                        out=x3[0:1, 0:PX, PX : W + PX], in_=cpad[0:1, 0 : 2 * W]
                    )
                    nc.sync.dma_start(
                        out=x3[127:128, KR + PX : SEGS, PX : W + PX],
                        in_=cpad[0:1, 0 : 2 * W],
                    )

                    # main rows: partition p segs 2..9 <- rows 8p..8p+7
                    nc.sync.dma_start(
                        out=x3[:, PX : PX + KR, PX : W + PX],
                        in_=x[img].rearrange("(p s) c -> p s c", s=KR),
                    )
                    # halo segs via strided row sampling
                    nc.sync.dma_start(
                        out=x3[1:128, 0:1, PX : W + PX],
                        in_=x[img, KR - PX : H - PX : KR, :].unsqueeze(1),
                    )
                    nc.sync.dma_start(
                        out=x3[1:128, 1:2, PX : W + PX],
                        in_=x[img, KR - 1 : H - 1 : KR, :].unsqueeze(1),
                    )
                    nc.sync.dma_start(
                        out=x3[0:127, KR + PX : KR + PX + 1, PX : W + PX],
                        in_=x[img, KR:H:KR, :].unsqueeze(1),
                    )
                    nc.sync.dma_start(
                        out=x3[0:127, KR + PX + 1 : SEGS, PX : W + PX],
                        in_=x[img, KR + 1 : H : KR, :].unsqueeze(1),
                    )

                    of = opool.tile([128, KR * W], F32, tag="o")
                    of3 = of[:, :].rearrange("p (s c) -> p s c", s=KR)

                    for cc in range(N_CC):
                        c0 = cc * CW
                        xs = x3[:, :, c0 : c0 + CWH]

                        w2 = t1p.tile([128, (SEGS - 1) * CWH], F32, tag="t1")
                        w2_3 = w2[:, :].rearrange("p (s c) -> p s c", s=SEGS - 1)
                        nc.vector.tensor_tensor(
                            out=w2_3[:, :, :],
                            in0=xs[:, 0 : SEGS - 1, :],
                            in1=xs[:, 1:SEGS, :],
                            op=MIN,
                        )
                        w4 = t2p.tile([128, (SEGS - 3) * CWH], F32, tag="t2")
                        w4_3 = w4[:, :].rearrange("p (s c) -> p s c", s=SEGS - 3)
                        nc.vector.tensor_tensor(
                            out=w4_3[:, :, :],
                            in0=w2_3[:, 0 : SEGS - 3, :],
                            in1=w2_3[:, 2 : SEGS - 1, :],
                            op=MIN,
                        )
                        v = vpool.tile([128, KR * CWH], F32, tag="v")
                        v3 = v[:, :].rearrange("p (s c) -> p s c", s=KR)
                        nc.vector.tensor_tensor(
                            out=v3[:, :, :],
                            in0=w4_3[:, 0:KR, :],
                            in1=xs[:, 2 * PX : SEGS, :],
                            op=MIN,
                        )

                        a = t1p.tile([128, KR * (CWH - 1)], F32, tag="t1")
                        a3 = a[:, :].rearrange("p (s c) -> p s c", s=KR)
                        nc.vector.tensor_tensor(
                            out=a3[:, :, :],
                            in0=v3[:, :, 0 : CWH - 1],
                            in1=v3[:, :, 1:CWH],
                            op=MIN,
                        )
                        bb = t2p.tile([128, KR * (CWH - 3)], F32, tag="t2")
                        b3 = bb[:, :].rearrange("p (s c) -> p s c", s=KR)
                        nc.vector.tensor_tensor(
                            out=b3[:, :, :],
                            in0=a3[:, :, 0 : CWH - 3],
                            in1=a3[:, :, 2 : CWH - 1],
                            op=MIN,
                        )
                        nc.vector.tensor_tensor(
                            out=of3[:, :, c0 : c0 + CW],
                            in0=b3[:, :, 0:CW],
                            in1=v3[:, :, 2 * PX : CWH],
                            op=MIN,
                        )

                    nc.scalar.dma_start(
                        out=y[img].rearrange("(p s) c -> p s c", s=KR),
                        in_=of3[:, :, :],
                    )

    nc.compile()
    return nc


def run(mask: np.ndarray, trace: bool = False):
    assert mask.shape == (B, 1, H, W), mask.shape
    in_dtype = mask.dtype
    mask4 = np.ascontiguousarray(
        mask.reshape(B, H, W).astype(np.float32, copy=False)
    )
    if "nc" not in _CACHE:
        _CACHE["nc"] = build_nc(1)
    nc = _CACHE["nc"]
    in_maps = [
        {"mask": mask4[i * PER_CORE : (i + 1) * PER_CORE]} for i in range(N_CORES)
    ]
    res = run_bass_kernel_spmd(nc, in_maps, list(range(N_CORES)), trace=trace)
    out = np.concatenate([res.results[i]["out"] for i in range(N_CORES)], axis=0)
    return out.reshape(B, 1, H, W).astype(in_dtype, copy=False), res


def kernel(mask: np.ndarray) -> np.ndarray:
    return run(mask)[0]



# revision 2
# speedup vs baseline: 1.0329x; 1.0329x over previous
"""Erosion (5x5 sliding-window min, geodesic border pad 1e4) on TRN2.

Layout: partition p holds rows 8p-2 .. 8p+9 of one image as 12
contiguous 1024-col bf16 segments, loaded in ONE overlapping-window
SWDGE cast-DMA (f32 DRAM -> bf16 SBUF, one 48KB-read descriptor per
partition; partitions 0/127 get clamped single-descriptor loads, pad
rows come from a 1e4 const tile). Cast-DMA measured at ~340 GB/s with
a ~0.8us trigger, vs ~94us/image of HWDGE descriptor-generation stall
for the old 4KB-run layout.

Compute is full-width bf16 on DVE (2 elem/cycle/lane, ~0.53ns/elem
per partition, shifted operands included): vertical min = w2/w4/v
cascade along the segment axis (w4 in-place into w2, 2-seg read-ahead
is pipeline-safe), horizontal min = a/b cascade plus 4 one-column edge
TTs for the geodesic border (no column pads needed). Store is one
SWDGE cast-DMA bf16 -> f32 per image (128 x 32KB contiguous).

bf16 rounding keeps rel err ~2^-9 (tolerance 2e-2). DVE is the
critical path at ~29us/image; loads/stores overlap via double-buffered
x/out tiles.
"""

import numpy as np

import concourse.bacc as bacc
import concourse.mybir as mybir
import concourse.tile as tile
from concourse.bass import AP
from concourse.bass_utils import run_bass_kernel_spmd

B, H, W = 32, 1024, 1024
N_CORES = 8
PER_CORE = B // N_CORES     # 4 images per core
PX = 2
PAD_VAL = 1e4
F32 = mybir.dt.float32
BF16 = mybir.dt.bfloat16
MIN = mybir.AluOpType.min

KR = 8                      # output rows per partition (128*8 = 1024)
SEGS = KR + 2 * PX          # 12 segments per partition

_CACHE = {}


def build_nc(repeat: int = 1):
    nc = bacc.Bacc("TRN2", debug=False, num_devices=N_CORES)
    x = nc.dram_tensor("mask", [PER_CORE, H, W], F32, kind="ExternalInput").ap()
    y = nc.dram_tensor("out", [PER_CORE, H, W], F32, kind="ExternalOutput").ap()

    with tile.TileContext(nc) as tc:
        with (
            tc.tile_pool(name="const", bufs=1) as cpool,
            tc.tile_pool(name="xp", bufs=2) as xpool,
            tc.tile_pool(name="w2p", bufs=1) as w2pool,
            tc.tile_pool(name="vp", bufs=1) as vpool,
            tc.tile_pool(name="ap_", bufs=1) as apool,
            tc.tile_pool(name="bp", bufs=1) as bpool,
            tc.tile_pool(name="op", bufs=2) as opool,
        ):
            # 1e4 source for row-pad fills (memset can't start at
            # partition 127; DMA is exempt from start-partition rules)
            cpad = cpool.tile([128, 2 * W], BF16)
            nc.vector.memset(cpad[:, :], PAD_VAL)

            for rep in range(repeat):
                for img in range(PER_CORE):
                    xb = xpool.tile([128, SEGS * W], BF16, tag="x")
                    x3 = xb[:, :].rearrange("p (s c) -> p s c", s=SEGS)

                    # main overlap load: partition p (1..126) <- rows
                    # 8p-2 .. 8p+9, one 48KB f32-read descriptor each,
                    # cast to bf16 on the way in (SWDGE only)
                    nc.gpsimd.dma_start(
                        out=xb[1:127, :],
                        in_=AP(
                            x.tensor,
                            img * H * W + (KR - PX) * W,
                            [[KR * W, 126], [1, SEGS * W]],
                        ),
                    )
                    # partition 0: rows 0..9 -> segs 2..11
                    nc.gpsimd.dma_start(
                        out=xb[0:1, PX * W : SEGS * W],
                        in_=AP(
                            x.tensor,
                            img * H * W,
                            [[(SEGS - PX) * W, 1], [1, (SEGS - PX) * W]],
                        ),
                    )
                    # partition 127: rows 1014..1023 -> segs 0..9
                    nc.gpsimd.dma_start(
                        out=xb[127:128, 0 : (SEGS - PX) * W],
                        in_=AP(
                            x.tensor,
                            img * H * W + (H - (SEGS - PX)) * W,
                            [[(SEGS - PX) * W, 1], [1, (SEGS - PX) * W]],
                        ),
                    )
                    # row pads: partition 0 segs 0,1 / partition 127 segs 10,11
                    nc.sync.dma_start(
                        out=xb[0:1, 0 : PX * W], in_=cpad[0:1, 0 : PX * W]
                    )
                    nc.sync.dma_start(
                        out=xb[127:128, (SEGS - PX) * W : SEGS * W],
                        in_=cpad[0:1, 0 : PX * W],
                    )

                    # vertical pass: w2[s]=min(x[s],x[s+1]) s=0..10,
                    # w4[s]=min(w2[s],w2[s+2]) s=0..8 (in-place),
                    # v[j]=min(w4[j], x[j+4]) j=0..7
                    w2 = w2pool.tile([128, (SEGS - 1) * W], BF16, tag="w2")
                    w2_3 = w2[:, :].rearrange("p (s c) -> p s c", s=SEGS - 1)
                    nc.vector.tensor_tensor(
                        out=w2_3[:, :, :],
                        in0=x3[:, 0 : SEGS - 1, :],
                        in1=x3[:, 1:SEGS, :],
                        op=MIN,
                    )
                    nc.vector.tensor_tensor(
                        out=w2_3[:, 0 : SEGS - 3, :],
                        in0=w2_3[:, 0 : SEGS - 3, :],
                        in1=w2_3[:, 2 : SEGS - 1, :],
                        op=MIN,
                    )
                    v = vpool.tile([128, KR * W], BF16, tag="v")
                    v3 = v[:, :].rearrange("p (s c) -> p s c", s=KR)
                    nc.vector.tensor_tensor(
                        out=v3[:, :, :],
                        in0=w2_3[:, 0:KR, :],
                        in1=x3[:, 2 * PX : SEGS, :],
                        op=MIN,
                    )

                    # horizontal pass (full width, no column pads):
                    # a[c]=min(v[c],v[c+1]) c=0..1022
                    # b[c]=min(a[c],a[c+2]) c=0..1020
                    # out[c]=min(b[c-2], v[c+2]) c=2..1021
                    # edges: out[0]=min(a0,a1), out[1]=min(b0,v0),
                    #        out[1022]=min(b1020,v1022),
                    #        out[1023]=min(a1021,a1022)
                    aa = apool.tile([128, KR * W], BF16, tag="a")
                    a3 = aa[:, :].rearrange("p (s c) -> p s c", s=KR)
                    nc.vector.tensor_tensor(
                        out=a3[:, :, 0 : W - 1],
                        in0=v3[:, :, 0 : W - 1],
                        in1=v3[:, :, 1:W],
                        op=MIN,
                    )
                    bb = bpool.tile([128, KR * W], BF16, tag="b")
                    b3 = bb[:, :].rearrange("p (s c) -> p s c", s=KR)
                    nc.vector.tensor_tensor(
                        out=b3[:, :, 0 : W - 3],
                        in0=a3[:, :, 0 : W - 3],
                        in1=a3[:, :, 2 : W - 1],
                        op=MIN,
                    )
                    ob = opool.tile([128, KR * W], BF16, tag="o")
                    o3 = ob[:, :].rearrange("p (s c) -> p s c", s=KR)
                    nc.vector.tensor_tensor(
                        out=o3[:, :, PX : W - PX],
                        in0=b3[:, :, 0 : W - 2 * PX],
                        in1=v3[:, :, 2 * PX : W],
                        op=MIN,
                    )
                    nc.vector.tensor_tensor(
                        out=o3[:, :, 0:1],
                        in0=a3[:, :, 0:1],
                        in1=a3[:, :, 1:2],
                        op=MIN,
                    )
                    nc.vector.tensor_tensor(
                        out=o3[:, :, 1:2],
                        in0=b3[:, :, 0:1],
                        in1=v3[:, :, 0:1],
                        op=MIN,
                    )
                    nc.vector.tensor_tensor(
                        out=o3[:, :, W - 2 : W - 1],
                        in0=b3[:, :, W - 4 : W - 3],
                        in1=v3[:, :, W - 2 : W - 1],
                        op=MIN,
                    )
                    nc.vector.tensor_tensor(
                        out=o3[:, :, W - 1 : W],
                        in0=a3[:, :, W - 3 : W - 2],
                        in1=a3[:, :, W - 2 : W - 1],
                        op=MIN,
                    )

                    # store: bf16 -> f32 cast during DMA (SWDGE),
                    # 128 x 32KB contiguous descriptors
                    nc.gpsimd.dma_start(
                        out=y[img].rearrange("(p s) c -> p (s c)", s=KR),
                        in_=ob[:, :],
                    )

    nc.compile()
    return nc


def run(mask: np.ndarray, trace: bool = False, tmpdir: str | None = None):
    assert mask.shape == (B, 1, H, W), mask.shape
    in_dtype = mask.dtype
    mask4 = np.ascontiguousarray(
        mask.reshape(B, H, W).astype(np.float32, copy=False)
    )
    if "nc" not in _CACHE:
        _CACHE["nc"] = build_nc(1)
    nc = _CACHE["nc"]
    in_maps = [
        {"mask": mask4[i * PER_CORE : (i + 1) * PER_CORE]} for i in range(N_CORES)
    ]
    res = run_bass_kernel_spmd(
        nc, in_maps, list(range(N_CORES)), trace=trace, tmpdir=tmpdir
    )
    out = np.concatenate([res.results[i]["out"] for i in range(N_CORES)], axis=0)
    return out.reshape(B, 1, H, W).astype(in_dtype, copy=False), res


def kernel(mask: np.ndarray) -> np.ndarray:
    return run(mask)[0]


# revision 6
# speedup vs baseline: 4.6814x; 4.5322x over previous
"""Erosion (5x5 sliding-window min, geodesic border pad 1e4) on TRN2.

Layout: partition p holds rows 8p-2 .. 8p+9 of one image as 12
contiguous 1024-col bf16 segments, loaded via overlapping-window SWDGE
cast-DMAs (f32 DRAM -> bf16 SBUF, one fat contiguous-read descriptor
per partition; partitions 0/127 get clamped single-descriptor loads,
pad rows come from a 1e4 const tile, pre-filled once per x buffer
since no load ever touches the pad regions). Cast-DMA measured at
~340 GB/s with ~0.8us triggers, vs ~94us/image of HWDGE
descriptor-generation stall for the old 4KB-run layout.

Compute is full-width bf16 on DVE (2 elem/cycle/lane, ~0.53ns/elem
per partition, shifted operands included): vertical min = w2/w4/v
cascade along the segment axis (w4 in-place into w2; in1 reads run
2 segments ahead of the write pointer - pipeline-safe), horizontal
min = a/b cascade plus one-column edge TTs for the geodesic border
(no column pads needed). Store casts bf16 -> f32 in the DMA (SWDGE),
contiguous per-partition descriptors.

The image is processed in TWO 4-output-seg groups to shorten the
pipeline fill (group 1 needs only input segs 0..7 = the first load)
and drain (first half stores while the second half computes). bf16
rounding keeps rel err ~2e-3 (tolerance 2e-2). DVE is the critical
path at ~29us/image.
"""

import numpy as np

import concourse.bacc as bacc
import concourse.mybir as mybir
import concourse.tile as tile
from concourse.bass import AP
from concourse.bass_utils import run_bass_kernel_spmd

B, H, W = 32, 1024, 1024
N_CORES = 8
PER_CORE = B // N_CORES     # 4 images per core
PX = 2
PAD_VAL = 1e4
F32 = mybir.dt.float32
BF16 = mybir.dt.bfloat16
MIN = mybir.AluOpType.min

KR = 8                      # output rows per partition (128*8 = 1024)
SEGS = KR + 2 * PX          # 12 segments per partition
GA = 8                      # input segs loaded in group A (segs 0..7)

_CACHE = {}


def build_nc(repeat: int = 1):
    nc = bacc.Bacc("TRN2", debug=False, num_devices=N_CORES)
    x = nc.dram_tensor("mask", [PER_CORE, H, W], F32, kind="ExternalInput").ap()
    y = nc.dram_tensor("out", [PER_CORE, H, W], F32, kind="ExternalOutput").ap()

    with tile.TileContext(nc) as tc:
        with (
            tc.tile_pool(name="const", bufs=1) as cpool,
            tc.tile_pool(name="xp", bufs=1) as xpool,
            tc.tile_pool(name="wp", bufs=1) as wpool,
            tc.tile_pool(name="op", bufs=1) as opool,
        ):
            # 1e4 source for row-pad fills (memset can't start at
            # partition 127; DMA is exempt from start-partition rules)
            cpad = cpool.tile([128, PX * W], BF16)
            nc.vector.memset(cpad[:, :], PAD_VAL)

            # manual double-buffers; pad regions (p0 segs 0,1 and p127
            # segs 10,11) are written ONLY here, so fill them once
            xbufs, obufs = [], []
            for i in range(2):
                xb = xpool.tile([128, SEGS * W], BF16, tag=f"x{i}")
                nc.sync.dma_start(out=xb[0:1, 0 : PX * W], in_=cpad[0:1, :])
                nc.sync.dma_start(
                    out=xb[127:128, (SEGS - PX) * W : SEGS * W], in_=cpad[0:1, :]
                )
                xbufs.append(xb)
                obufs.append(
                    opool.tile([128, KR * W], BF16, tag=f"o{i}", name=f"ob{i}")
                )
            w2 = wpool.tile([128, (SEGS - 1) * W], BF16, tag="w2")
            w2_3 = w2[:, :].rearrange("p (s c) -> p s c", s=SEGS - 1)
            v = wpool.tile([128, KR * W], BF16, tag="v")
            v3 = v[:, :].rearrange("p (s c) -> p s c", s=KR)
            aa = wpool.tile([128, KR * W], BF16, tag="a")
            a3 = aa[:, :].rearrange("p (s c) -> p s c", s=KR)
            bb = wpool.tile([128, KR * W], BF16, tag="b")
            b3 = bb[:, :].rearrange("p (s c) -> p s c", s=KR)

            for rep in range(repeat):
                for img in range(PER_CORE):
                    xb = xbufs[img % 2]
                    ob = obufs[img % 2]
                    x3 = xb[:, :].rearrange("p (s c) -> p s c", s=SEGS)
                    o3 = ob[:, :].rearrange("p (s c) -> p s c", s=KR)

                    # ---- loads (SWDGE cast f32->bf16) ----
                    # group A: segs 0..7 = rows 8p-2 .. 8p+5 (p 1..126)
                    nc.gpsimd.dma_start(
                        out=xb[1:127, 0 : GA * W],
                        in_=AP(
                            x.tensor,
                            img * H * W + (KR - PX) * W,
                            [[KR * W, 126], [1, GA * W]],
                        ),
                    )
                    # p0: segs 2..11 <- rows 0..9 (one descriptor)
                    nc.gpsimd.dma_start(
                        out=xb[0:1, PX * W : SEGS * W],
                        in_=AP(
                            x.tensor,
                            img * H * W,
                            [[(SEGS - PX) * W, 1], [1, (SEGS - PX) * W]],
                        ),
                    )
                    # p127: segs 0..9 <- rows 1014..1023
                    nc.gpsimd.dma_start(
                        out=xb[127:128, 0 : (SEGS - PX) * W],
                        in_=AP(
                            x.tensor,
                            img * H * W + (H - (SEGS - PX)) * W,
                            [[(SEGS - PX) * W, 1], [1, (SEGS - PX) * W]],
                        ),
                    )
                    # group B: segs 8..11 = rows 8p+6 .. 8p+9 (p 1..126)
                    nc.gpsimd.dma_start(
                        out=xb[1:127, GA * W : SEGS * W],
                        in_=AP(
                            x.tensor,
                            img * H * W + (KR - PX + GA) * W,
                            [[KR * W, 126], [1, (SEGS - GA) * W]],
                        ),
                    )

                    # ---- compute + store in two 4-output-seg groups ----
                    # group 1 (out segs 0..3) depends only on x segs 0..7
                    # group 2 (out segs 4..7) additionally on segs 8..11
                    for g in range(2):
                        s0 = 4 * g          # first output seg of group
                        if g == 0:
                            # w2[0..6], w4[0..4] (one extra for group 2)
                            nc.vector.tensor_tensor(
                                out=w2_3[:, 0:7, :],
                                in0=x3[:, 0:7, :],
                                in1=x3[:, 1:8, :],
                                op=MIN,
                            )
                            nc.vector.tensor_tensor(
                                out=w2_3[:, 0:5, :],
                                in0=w2_3[:, 0:5, :],
                                in1=w2_3[:, 2:7, :],
                                op=MIN,
                            )
                            nc.vector.tensor_tensor(
                                out=v3[:, 0:4, :],
                                in0=w2_3[:, 0:4, :],
                                in1=x3[:, 4:8, :],
                                op=MIN,
                            )
                        else:
                            # w2[7..9], w4[5..7], v[4..7]
                            # (w2[10]/w4[8] are never consumed:
                            #  v[7] = min(w4[7], x[11]) covers segs 7..11)
                            nc.vector.tensor_tensor(
                                out=w2_3[:, 7:10, :],
                                in0=x3[:, 7:10, :],
                                in1=x3[:, 8:11, :],
                                op=MIN,
                            )
                            nc.vector.tensor_tensor(
                                out=w2_3[:, 5:8, :],
                                in0=w2_3[:, 5:8, :],
                                in1=w2_3[:, 7:10, :],
                                op=MIN,
                            )
                            nc.vector.tensor_tensor(
                                out=v3[:, 4:8, :],
                                in0=w2_3[:, 4:8, :],
                                in1=x3[:, 8:12, :],
                                op=MIN,
                            )

                        sl = slice(s0, s0 + 4)
                        # horizontal cascade on this group's 4 segs
                        nc.vector.tensor_tensor(
                            out=a3[:, sl, 0 : W - 1],
                            in0=v3[:, sl, 0 : W - 1],
                            in1=v3[:, sl, 1:W],
                            op=MIN,
                        )
                        nc.vector.tensor_tensor(
                            out=b3[:, sl, 0 : W - 3],
                            in0=a3[:, sl, 0 : W - 3],
                            in1=a3[:, sl, 2 : W - 1],
                            op=MIN,
                        )
                        nc.vector.tensor_tensor(
                            out=o3[:, sl, PX : W - PX],
                            in0=b3[:, sl, 0 : W - 2 * PX],
                            in1=v3[:, sl, 2 * PX : W],
                            op=MIN,
                        )
                        # geodesic edge columns
                        nc.vector.tensor_tensor(
                            out=o3[:, sl, 0:1],
                            in0=a3[:, sl, 0:1],
                            in1=a3[:, sl, 1:2],
                            op=MIN,
                        )
                        nc.vector.tensor_tensor(
                            out=o3[:, sl, 1:2],
                            in0=b3[:, sl, 0:1],
                            in1=v3[:, sl, 0:1],
                            op=MIN,
                        )
                        nc.vector.tensor_tensor(
                            out=o3[:, sl, W - 2 : W - 1],
                            in0=b3[:, sl, W - 4 : W - 3],
                            in1=v3[:, sl, W - 2 : W - 1],
                            op=MIN,
                        )
                        nc.vector.tensor_tensor(
                            out=o3[:, sl, W - 1 : W],
                            in0=a3[:, sl, W - 3 : W - 2],
                            in1=a3[:, sl, W - 2 : W - 1],
                            op=MIN,
                        )
                        # store this half: bf16 -> f32 cast in the DMA,
                        # 128 x 16KB contiguous descriptors
                        # (partition p -> rows 8p+s0 .. 8p+s0+3)
                        nc.gpsimd.dma_start(
                            out=AP(
                                y.tensor,
                                img * H * W + s0 * W,
                                [[KR * W, 128], [1, 4 * W]],
                            ),
                            in_=ob[:, s0 * W : (s0 + 4) * W],
                        )

    nc.compile()
    return nc


def run(mask: np.ndarray, trace: bool = False, tmpdir: str | None = None):
    assert mask.shape == (B, 1, H, W), mask.shape
    in_dtype = mask.dtype
    mask4 = np.ascontiguousarray(
        mask.reshape(B, H, W).astype(np.float32, copy=False)
    )
    if "nc" not in _CACHE:
        _CACHE["nc"] = build_nc(1)
    nc = _CACHE["nc"]
    in_maps = [
        {"mask": mask4[i * PER_CORE : (i + 1) * PER_CORE]} for i in range(N_CORES)
    ]
    res = run_bass_kernel_spmd(
        nc, in_maps, list(range(N_CORES)), trace=trace, tmpdir=tmpdir
    )
    out = np.concatenate([res.results[i]["out"] for i in range(N_CORES)], axis=0)
    return out.reshape(B, 1, H, W).astype(in_dtype, copy=False), res


def kernel(mask: np.ndarray) -> np.ndarray:
    return run(mask)[0]


# revision 7
# speedup vs baseline: 9.0362x; 1.9302x over previous
"""Erosion (5x5 sliding-window min, geodesic border pad 1e4) on TRN2.

Layout: partition p holds rows 8p-2 .. 8p+9 of one image as 12
contiguous 1024-col bf16 segments, loaded via overlapping-window SWDGE
cast-DMAs (f32 DRAM -> bf16 SBUF, one fat contiguous-read descriptor
per partition; partitions 0/127 get clamped single-descriptor loads,
pad rows come from a 1e4 const tile, pre-filled once per x buffer
since no load ever touches the pad regions). Cast-DMA measured at
~340 GB/s with ~0.8us triggers, vs ~94us/image of HWDGE
descriptor-generation stall for a 4KB-run layout.

Compute is full-width bf16 on DVE (2 elem/cycle/lane, ~0.53ns/elem per
partition, shifted operands included): vertical min = w2/w4/v cascade
along the segment axis (w4 in-place into w2; in1 reads run 2 segments
ahead of the write pointer - pipeline-safe), horizontal min = a/b
cascade plus one-column edge TTs for the geodesic border (no column
pads). Stores cast bf16 -> f32 in the DMA (SWDGE), contiguous
per-partition descriptors.

Pipelining: the GpSimd (SWDGE) queue is IN-ORDER, so a store trigger
that waits on compute would block later load triggers. Issue order is
software-pipelined: loads for stream position k+2 are issued after the
stores of position k; 3 x-buffers / 2 out-buffers. Each image computes
in two 4-output-seg groups (group 1 needs only input segs 0..7); the
first image's segs 0..7 load is further split for a shorter pipeline
fill, the last image's second half stores in two 2-seg pieces for a
shorter drain. bf16 rounding keeps rel err ~2e-3 (tolerance 2e-2).
DVE is the critical path at ~28us/image.
"""

import numpy as np

import concourse.bacc as bacc
import concourse.mybir as mybir
import concourse.tile as tile
from concourse.bass import AP
from concourse.bass_utils import run_bass_kernel_spmd

B, H, W = 32, 1024, 1024
N_CORES = 8
PER_CORE = B // N_CORES     # 4 images per core
PX = 2
PAD_VAL = 1e4
F32 = mybir.dt.float32
BF16 = mybir.dt.bfloat16
MIN = mybir.AluOpType.min

KR = 8                      # output rows per partition (128*8 = 1024)
SEGS = KR + 2 * PX          # 12 segments per partition
GA = 8                      # input segs needed by compute group 1

_CACHE = {}


def build_nc(repeat: int = 1):
    nc = bacc.Bacc("TRN2", debug=False, num_devices=N_CORES)
    x = nc.dram_tensor("mask", [PER_CORE, H, W], F32, kind="ExternalInput").ap()
    y = nc.dram_tensor("out", [PER_CORE, H, W], F32, kind="ExternalOutput").ap()

    N = repeat * PER_CORE   # flat image stream

    with tile.TileContext(nc) as tc:
        with (
            tc.tile_pool(name="const", bufs=1) as cpool,
            tc.tile_pool(name="xp", bufs=1) as xpool,
            tc.tile_pool(name="wp", bufs=1) as wpool,
            tc.tile_pool(name="op", bufs=1) as opool,
        ):
            # 1e4 source for row-pad fills (memset can't start at
            # partition 127; DMA is exempt from start-partition rules)
            cpad = cpool.tile([128, PX * W], BF16)
            nc.vector.memset(cpad[:, :], PAD_VAL)

            # manual buffers; pad regions (p0 segs 0,1 / p127 segs
            # 10,11) are written ONLY here, so fill them once
            xbufs, obufs = [], []
            for i in range(3):
                xb = xpool.tile([128, SEGS * W], BF16, tag=f"x{i}", name=f"xb{i}")
                nc.sync.dma_start(out=xb[0:1, 0 : PX * W], in_=cpad[0:1, :])
                nc.sync.dma_start(
                    out=xb[127:128, (SEGS - PX) * W : SEGS * W], in_=cpad[0:1, :]
                )
                xbufs.append(xb)
            for i in range(2):
                obufs.append(
                    opool.tile([128, KR * W], BF16, tag=f"o{i}", name=f"ob{i}")
                )
            w2 = wpool.tile([128, (SEGS - 2) * W], BF16, tag="w2")
            w2_3 = w2[:, :].rearrange("p (s c) -> p s c", s=SEGS - 2)
            v = wpool.tile([128, KR * W], BF16, tag="v")
            v3 = v[:, :].rearrange("p (s c) -> p s c", s=KR)
            aa = wpool.tile([128, KR * W], BF16, tag="a")
            a3 = aa[:, :].rearrange("p (s c) -> p s c", s=KR)
            bb = wpool.tile([128, KR * W], BF16, tag="b")
            b3 = bb[:, :].rearrange("p (s c) -> p s c", s=KR)

            def issue_loads(k):
                """SWDGE cast loads (f32->bf16) for stream position k."""
                img = k % PER_CORE
                xb = xbufs[k % 3]
                base = img * H * W
                # p0: segs 2..11 <- rows 0..9 (one descriptor)
                nc.gpsimd.dma_start(
                    out=xb[0:1, PX * W : SEGS * W],
                    in_=AP(
                        x.tensor, base, [[(SEGS - PX) * W, 1], [1, (SEGS - PX) * W]]
                    ),
                )
                # p127: segs 0..9 <- rows 1014..1023
                nc.gpsimd.dma_start(
                    out=xb[127:128, 0 : (SEGS - PX) * W],
                    in_=AP(
                        x.tensor,
                        base + (H - (SEGS - PX)) * W,
                        [[(SEGS - PX) * W, 1], [1, (SEGS - PX) * W]],
                    ),
                )
                # main overlap loads, partitions 1..126: segs 0..7 then
                # 8..11 (rows 8p-2+s); first image splits segs 0..7 in
                # two for a shorter pipeline fill
                halves = [(0, 4), (4, 4)] if k == 0 else [(0, GA)]
                for lo, n in halves:
                    nc.gpsimd.dma_start(
                        out=xb[1:127, lo * W : (lo + n) * W],
                        in_=AP(
                            x.tensor,
                            base + (KR - PX + lo) * W,
                            [[KR * W, 126], [1, n * W]],
                        ),
                    )
                nc.gpsimd.dma_start(
                    out=xb[1:127, GA * W : SEGS * W],
                    in_=AP(
                        x.tensor,
                        base + (KR - PX + GA) * W,
                        [[KR * W, 126], [1, (SEGS - GA) * W]],
                    ),
                )

            def horizontal(o3, sl):
                """a/b cascade + geodesic edge columns for out segs sl."""
                nc.vector.tensor_tensor(
                    out=a3[:, sl, 0 : W - 1],
                    in0=v3[:, sl, 0 : W - 1],
                    in1=v3[:, sl, 1:W],
                    op=MIN,
                )
                nc.vector.tensor_tensor(
                    out=b3[:, sl, 0 : W - 3],
                    in0=a3[:, sl, 0 : W - 3],
                    in1=a3[:, sl, 2 : W - 1],
                    op=MIN,
                )
                nc.vector.tensor_tensor(
                    out=o3[:, sl, PX : W - PX],
                    in0=b3[:, sl, 0 : W - 2 * PX],
                    in1=v3[:, sl, 2 * PX : W],
                    op=MIN,
                )
                nc.vector.tensor_tensor(
                    out=o3[:, sl, 0:1], in0=a3[:, sl, 0:1], in1=a3[:, sl, 1:2], op=MIN
                )
                nc.vector.tensor_tensor(
                    out=o3[:, sl, 1:2], in0=b3[:, sl, 0:1], in1=v3[:, sl, 0:1], op=MIN
                )
                nc.vector.tensor_tensor(
                    out=o3[:, sl, W - 2 : W - 1],
                    in0=b3[:, sl, W - 4 : W - 3],
                    in1=v3[:, sl, W - 2 : W - 1],
                    op=MIN,
                )
                nc.vector.tensor_tensor(
                    out=o3[:, sl, W - 1 : W],
                    in0=a3[:, sl, W - 3 : W - 2],
                    in1=a3[:, sl, W - 2 : W - 1],
                    op=MIN,
                )

            def store(k, s0, nseg):
                """SWDGE cast store (bf16->f32): out segs s0..s0+nseg-1,
                partition p -> rows 8p+s0 .. (contiguous descriptors)."""
                img = k % PER_CORE
                ob = obufs[k % 2]
                nc.gpsimd.dma_start(
                    out=AP(
                        y.tensor,
                        img * H * W + s0 * W,
                        [[KR * W, 128], [1, nseg * W]],
                    ),
                    in_=ob[:, s0 * W : (s0 + nseg) * W],
                )

            # prologue: loads for the first two stream positions
            issue_loads(0)
            if N > 1:
                issue_loads(1)

            for k in range(N):
                xb = xbufs[k % 3]
                ob = obufs[k % 2]
                x3 = xb[:, :].rearrange("p (s c) -> p s c", s=SEGS)
                o3 = ob[:, :].rearrange("p (s c) -> p s c", s=KR)

                # ---- group 1: out segs 0..3 (needs x segs 0..7) ----
                # w2[s]=min(x[s],x[s+1]); split w2 on the first image to
                # start after the first quarter-load
                if k == 0:
                    nc.vector.tensor_tensor(
                        out=w2_3[:, 0:3, :], in0=x3[:, 0:3, :], in1=x3[:, 1:4, :],
                        op=MIN,
                    )
                    nc.vector.tensor_tensor(
                        out=w2_3[:, 3:7, :], in0=x3[:, 3:7, :], in1=x3[:, 4:8, :],
                        op=MIN,
                    )
                else:
                    nc.vector.tensor_tensor(
                        out=w2_3[:, 0:7, :], in0=x3[:, 0:7, :], in1=x3[:, 1:8, :],
                        op=MIN,
                    )
                # w4[0..4] in place (one extra for group 2's v[4])
                nc.vector.tensor_tensor(
                    out=w2_3[:, 0:5, :], in0=w2_3[:, 0:5, :], in1=w2_3[:, 2:7, :],
                    op=MIN,
                )
                nc.vector.tensor_tensor(
                    out=v3[:, 0:4, :], in0=w2_3[:, 0:4, :], in1=x3[:, 4:8, :],
                    op=MIN,
                )
                horizontal(o3, slice(0, 4))
                store(k, 0, 4)

                # ---- group 2: out segs 4..7 (adds x segs 8..11) ----
                # w2[7..9], w4[5..7], v[4..7]; w2[10]/w4[8] are never
                # consumed (v[7] = min(w4[7], x[11]) covers segs 7..11)
                nc.vector.tensor_tensor(
                    out=w2_3[:, 7:10, :], in0=x3[:, 7:10, :], in1=x3[:, 8:11, :],
                    op=MIN,
                )
                nc.vector.tensor_tensor(
                    out=w2_3[:, 5:8, :], in0=w2_3[:, 5:8, :], in1=w2_3[:, 7:10, :],
                    op=MIN,
                )
                nc.vector.tensor_tensor(
                    out=v3[:, 4:8, :], in0=w2_3[:, 4:8, :], in1=x3[:, 8:12, :],
                    op=MIN,
                )
                if k == N - 1:
                    # shorter drain: two 2-seg pieces
                    horizontal(o3, slice(4, 6))
                    store(k, 4, 2)
                    horizontal(o3, slice(6, 8))
                    store(k, 6, 2)
                else:
                    horizontal(o3, slice(4, 8))
                    store(k, 4, 4)

                # loads for stream position k+2 AFTER this image's
                # stores (keeps the in-order SWDGE queue flowing)
                if k + 2 < N:
                    issue_loads(k + 2)

    nc.compile()
    return nc


def run(mask: np.ndarray, trace: bool = False, tmpdir: str | None = None):
    assert mask.shape == (B, 1, H, W), mask.shape
    in_dtype = mask.dtype
    mask4 = np.ascontiguousarray(
        mask.reshape(B, H, W).astype(np.float32, copy=False)
    )
    if "nc" not in _CACHE:
        _CACHE["nc"] = build_nc(1)
    nc = _CACHE["nc"]
    in_maps = [
        {"mask": mask4[i * PER_CORE : (i + 1) * PER_CORE]} for i in range(N_CORES)
    ]
    res = run_bass_kernel_spmd(
        nc, in_maps, list(range(N_CORES)), trace=trace, tmpdir=tmpdir
    )
    out = np.concatenate([res.results[i]["out"] for i in range(N_CORES)], axis=0)
    return out.reshape(B, 1, H, W).astype(in_dtype, copy=False), res


def kernel(mask: np.ndarray) -> np.ndarray:
    return run(mask)[0]
